# revision 1
# baseline (speedup 1.0000x reference)
"""Causal self-attention Trainium2 kernel (v2, bf16).

Full-model shapes: x [4, 2048, 1024], w_qkv [1024, 3072], b_qkv [3072],
w_out [1024, 1024], b_out [1024].  H=16 heads, D=64.

Sharding: 8 cores = 4 batches x 2 head-groups (tensor parallel).  Each core
computes qkv projection for its 8 heads of its batch, causal attention, and
the partial out-projection (512 of 1024 contraction rows).  The two partials
per batch are summed on the host (the "all-reduce" after out_proj), plus
b_out.

v2 design (vs the fp32r v1 baseline at ~410us):
  - All matmul operands bf16 (1 cycle/row like fp32r, but half the DMA and
    SBUF traffic, DVE 2x modes, no N<256 fp32r penalty).  PSUM stays f32.
  - Single merged phase A over all 8 heads: x is streamed from DRAM once
    (v1 streamed it twice, once per 4-head group).
  - Softmax exp emitted in [128, <=1024] windows (one ACTIVATE per
    (s-block, t-half)) to amortize the ~370ns/instruction Activation-engine
    overhead; phase B is exp-throughput-bound on the Scalar engine.
  - Deferred 1/Z normalization per (head, 512-quarter), pipelined behind the
    next head's attention: DVE reciprocal + GpSimd partition_broadcast +
    fused mul into the yT copy.  v1 batched this per 4-head group and
    stalled the PE ~26us.
  - Out-projection PSUM gets its own 2 banks so phase C m-tiles interleave
    into the last head's attention stream; output DMA is spread across C.
  - Weights are host-packed so every weight DMA is contiguous.

Layout per core:
  - xT [C, T] bf16 host-transposed; streamed as [128, KT, chunk] tiles.
  - qT [feat, T] (feature-on-partition, 2 heads per 128-tile), kz per-head
    K-padded [128, T] (64 rows k_h + 64 zero rows -> S matmuls contract over
    full 128 partitions).
  - v natural [T, feat] with a ones column per head so PV yields y_un and
    the softmax denominator Z in one PSUM accumulation.
  - scores S^T [s, t] per (s-block, t-half window); diagonal blocks masked
    alternately pre-exp additive (DVE) / post-exp multiplicative (GpSimd).
"""

import sys
from contextlib import ExitStack

import numpy as np

sys.path.insert(0, "/opt/trn_rl_repo")

import ml_dtypes

import concourse.bacc as bacc
import concourse.bass as bass
import concourse.tile as tile
from concourse import mybir
from concourse.bass_utils import run_bass_kernel_spmd

F32 = mybir.dt.float32
F32R = mybir.dt.float32r
BF16 = mybir.dt.bfloat16
BF = ml_dtypes.bfloat16

B, T, C, H = 4, 2048, 1024, 16
D = C // H  # 64
N_CORES = 8
HL = H // 2      # heads per core = 8
FL = HL * D      # local features = 512
KT = C // 128    # 8 contraction tiles


# debug bisect flags
C_INTERLEAVE = True   # emit out-proj m-tiles inside the last head's stream
EXP_W = 1024          # exp window width (1024 = cross-bank ACT reads)
DIAG_ALT = True       # alternate DVE-additive / GpSimd-mult diagonal masks


def _chunks(t0, tend, grid=512):
    """Aligned chunks [c0, c0+w) covering [t0, tend), clipped to the global
    `grid` so no chunk crosses a grid (= PSUM bank) boundary."""
    out = []
    while t0 < tend:
        w = min(grid - (t0 % grid), tend - t0)
        out.append((t0, w))
        t0 += w
    return out


def build_program(t_len=T):
    nc = bacc.Bacc(None, target_bir_lowering=False, debug=False)
    TT = t_len
    n_tt = TT // 128

    xT = nc.declare_dram_parameter("xT", [C, TT], BF16, isOutput=False)
    # host-packed: wqk[p, m, k*128+f] = w_m[k*128+p, f]; m 0-3 q, 4-7 k tiles
    wqk = nc.declare_dram_parameter("wqk", [128, 8, KT * 128], BF16,
                                    isOutput=False)
    wv = nc.declare_dram_parameter("wv", [128, KT * FL], BF16, isOutput=False)
    wout = nc.declare_dram_parameter("wout", [FL, C], BF16, isOutput=False)
    bqk = nc.declare_dram_parameter("bqk", [128, 8], F32, isOutput=False)
    bv = nc.declare_dram_parameter("bv", [128, HL // 2], F32, isOutput=False)
    tri = nc.declare_dram_parameter("tri", [128, 128], BF16, isOutput=False)
    trineg = nc.declare_dram_parameter("trineg", [128, 128], F32,
                                       isOutput=False)
    out = nc.declare_dram_parameter("out", [TT, C], F32, isOutput=True)

    HW = min(1024, TT)   # t-half width for exp windows / S psum tiles
    QW = min(512, TT)    # y accumulation quarter width

    with tile.TileContext(nc) as tc, ExitStack() as top:
        persist = top.enter_context(tc.tile_pool(name="persist", bufs=1))
        stream = top.enter_context(tc.tile_pool(name="stream", bufs=2))
        upool = top.enter_context(tc.tile_pool(name="u", bufs=6))
        zpool = top.enter_context(tc.tile_pool(name="z", bufs=2))
        obpool = top.enter_context(tc.tile_pool(name="ob", bufs=3))

        wqk_sb = [persist.tile([128, KT * 128], BF16, tag=f"wqk{m}",
                               name=f"wqk{m}") for m in range(8)]
        wv_sb = persist.tile([128, KT * FL], BF16, tag="wv", name="wv_sb")
        qT_sb = [persist.tile([128, TT], BF16, tag=f"qT{j}", name=f"qT{j}")
                 for j in range(4)]
        kz_sb = [persist.tile([128, TT], BF16, tag=f"kz{h}", name=f"kz{h}")
                 for h in range(HL)]
        v_sb = persist.tile([128, n_tt, HL, D + 1], BF16, tag="v", name="v_sb")
        yT_sb = [persist.tile([128, TT], BF16, tag=f"yT{j}", name=f"yT{j}")
                 for j in range(4)]
        wout_sb = [persist.tile([128, C], BF16, tag=f"wo{j}", name=f"wo{j}")
                   for j in range(4)]
        bqk_sb = persist.tile([128, 8], F32, tag="bqk", name="bqk_sb")
        bv_sb = persist.tile([128, HL // 2], F32, tag="bv", name="bv_sb")
        tri_sb = persist.tile([128, 128], BF16, tag="tri", name="tri_sb")
        trineg_sb = persist.tile([128, 128], F32, tag="trineg",
                                 name="trineg_sb")

        # -------- phase A: qkv projection, all 8 heads, x streamed once ----
        if TT >= 1024:
            achunks = [(0, 256), (256, 256)] + [
                (c, 512) for c in range(512, TT, 512)]
        else:
            achunks = [(c, 256) for c in range(0, TT, 256)]
        xtiles = {}

        def load_chunk(ci):
            c0, ach = achunks[ci]
            xt = stream.tile([128, KT, 512], BF16, tag="x", name=f"x{ci}")
            for k in range(KT):
                nc.sync.dma_start(
                    out=xt[:, k, :ach],
                    in_=xT.rearrange("(k p) t -> p k t", p=128)[:, k,
                                                               c0:c0 + ach])
            xtiles[ci] = xt

        # first matmul needs wqk tile 0 + chunk 0: emit those DMAs first
        nc.sync.dma_start(out=wqk_sb[0], in_=wqk[:, 0, :])
        load_chunk(0)
        for m in range(1, 8):
            nc.sync.dma_start(out=wqk_sb[m], in_=wqk[:, m, :])
        nc.sync.dma_start(out=wv_sb, in_=wv[:])
        nc.sync.dma_start(out=bqk_sb, in_=bqk[:])
        nc.sync.dma_start(out=bv_sb, in_=bv[:])
        nc.sync.dma_start(out=tri_sb, in_=tri[:])
        nc.sync.dma_start(out=trineg_sb, in_=trineg[:])
        for h in range(HL):  # kz zero halves (the K-padding)
            zo = 64 * ((h + 1) % 2)
            nc.vector.memset(kz_sb[h][zo:zo + 64, :], 0.0)
        nc.vector.memset(v_sb[:, :, :, D], 1.0)  # PV ones column -> Z

        actx = ExitStack()
        a_pool = actx.enter_context(
            tc.tile_pool(name="a_psum", bufs=4, space="PSUM"))
        for ci, (c0, ach) in enumerate(achunks):
            if ci + 1 < len(achunks):
                load_chunk(ci + 1)
            xt = xtiles.pop(ci)
            for m in range(8):  # 4 q tiles then 4 k tiles
                ps = a_pool.tile([128, 512], F32, tag="mm", name="aps")
                for k in range(KT):
                    nc.tensor.matmul(
                        ps[:, :ach],
                        wqk_sb[m][:, k * 128:(k + 1) * 128],
                        xt[:, k, :ach],
                        start=(k == 0), stop=(k == KT - 1))
                bias = bqk_sb[:, m:m + 1]
                if m < 4:
                    nc.vector.tensor_scalar_add(
                        qT_sb[m][:, c0:c0 + ach], ps[:, :ach], bias)
                else:
                    # split k across the two per-head K-padded tiles
                    for par in range(2):
                        nc.vector.tensor_scalar_add(
                            kz_sb[(m - 4) * 2 + par][64 * par:64 * par + 64,
                                                     c0:c0 + ach],
                            ps[64 * par:64 * par + 64, :ach],
                            bias[64 * par:64 * par + 64, :])
            for sub in range(ach // 128):
                ps = a_pool.tile([128, 512], F32, tag="mm", name="aps")
                for k in range(KT):
                    nc.tensor.matmul(
                        ps[:, :FL],
                        xt[:, k, sub * 128:(sub + 1) * 128],
                        wv_sb[:, k * FL:(k + 1) * FL],
                        start=(k == 0), stop=(k == KT - 1))
                it = c0 // 128 + sub
                nc.vector.tensor_copy(
                    out=v_sb[:, it, :, 0:D],
                    in_=ps[:, :FL].rearrange("p (h d) -> p h d", h=HL))
        for j in range(4):  # w_out arrives during early phase B
            nc.sync.dma_start(out=wout_sb[j], in_=wout[j * 128:(j + 1) * 128, :])
        actx.close()

        # -------- phase B: attention per head; phase C interleaved ---------
        bctx = ExitStack()
        s_pool = bctx.enter_context(
            tc.tile_pool(name="s_psum", bufs=2, space="PSUM"))
        y_pool = bctx.enter_context(
            tc.tile_pool(name="y_psum", bufs=3, space="PSUM"))
        c_pool = bctx.enter_context(
            tc.tile_pool(name="c_psum", bufs=1, space="PSUM"))

        c_state = {"m": 0}

        def emit_c_tile():
            m = c_state["m"]
            c_state["m"] += 1
            ob = obpool.tile([128, C], F32, tag="ob", name="ob")
            for n in range(C // 512):
                ps = c_pool.tile([128, 512], F32, tag="c", name="cps")
                for j in range(4):
                    nc.tensor.matmul(
                        ps,
                        yT_sb[j][:, m * 128:(m + 1) * 128],
                        wout_sb[j][:, n * 512:(n + 1) * 512],
                        start=(j == 0), stop=(j == 3))
                nc.vector.tensor_copy(out=ob[:, n * 512:(n + 1) * 512], in_=ps)
            nc.sync.dma_start(out=out[m * 128:(m + 1) * 128, :], in_=ob)

        LAG = 5  # deferred PV chunks: hides exp latency from the PE
        n_half = TT // HW
        for h in range(HL):
            jt = h // 2           # q tile / yT tile index
            po = 64 * (h % 2)     # partition offset within the tile pair
            for half in range(n_half):
                h0 = half * HW
                yq = [y_pool.tile([D + 1, QW], F32, tag="y", name=f"yq{q}")
                      for q in range(HW // QW)]
                pend = []

                def emit_pv(s, c0, w, u):
                    q = (c0 - h0) // QW
                    nc.tensor.matmul(
                        yq[q][:, (c0 - h0) % QW:(c0 - h0) % QW + w],
                        v_sb[:, s, h, :],
                        u[:, c0 - h0:c0 - h0 + w],
                        start=(s == 0), stop=False,
                        skip_group_check=True)

                for s in range(n_tt):
                    if s * 128 >= h0 + HW:
                        break
                    t0 = max(s * 128, h0)
                    loc0 = t0 - h0
                    sps = s_pool.tile([128, HW], F32, tag="s", name="sps")
                    chunk_list = _chunks(t0, h0 + HW)
                    for (c0, w) in chunk_list:
                        nc.tensor.matmul(
                            sps[:, c0 - h0:c0 - h0 + w],
                            kz_sb[h][:, s * 128:(s + 1) * 128],
                            qT_sb[jt][:, c0:c0 + w],
                            start=True, stop=True,
                            skip_group_check=True)
                    diag = (s * 128 >= h0)
                    use_add = diag and (not DIAG_ALT or s % 2 == 0)
                    if use_add:  # pre-exp additive mask (DVE)
                        nc.vector.tensor_add(
                            sps[:, loc0:loc0 + 128], sps[:, loc0:loc0 + 128],
                            trineg_sb)
                    u = upool.tile([128, HW], BF16, tag="u", name="u")
                    for w0 in range(loc0 // EXP_W * EXP_W, HW, EXP_W):
                        a0, a1 = max(loc0, w0), min(w0 + EXP_W, HW)
                        nc.scalar.activation(
                            out=u[:, a0:a1], in_=sps[:, a0:a1],
                            func=mybir.ActivationFunctionType.Exp,
                            scale=1.0 / np.sqrt(D))
                    if diag and not use_add:  # post-exp mul mask (GpSimd)
                        nc.gpsimd.tensor_mul(
                            u[:, loc0:loc0 + 128], u[:, loc0:loc0 + 128],
                            tri_sb)
                    for (c0, w) in chunk_list:
                        pend.append((s, c0, w, u))
                    while len(pend) > LAG:
                        emit_pv(*pend.pop(0))
                    # interleave out-projection of the first half into the
                    # last head's second-half attention stream
                    if (C_INTERLEAVE and h == HL - 1 and half == n_half - 1
                            and s % 2 == 1
                            and c_state["m"] < (n_tt * (n_half - 1)) // n_half):
                        emit_c_tile()
                for p in pend:
                    emit_pv(*p)

                # deferred softmax normalization: 1/Z = exp(-ln Z) on the
                # Scalar engine (exp+ln share one activation table, so no
                # table swaps), broadcast from partition 0 (the only start
                # partition HW partition_broadcast honors), fused mul on DVE
                for q in range(HW // QW):
                    col0 = h0 + q * QW
                    lnz = zpool.tile([1, QW], F32, tag="lnz", name="lnz")
                    nc.scalar.activation(out=lnz, in_=yq[q][D:D + 1, :],
                                         func=mybir.ActivationFunctionType.Ln)
                    rz = zpool.tile([1, QW], F32, tag="rz", name="rz")
                    nc.scalar.activation(out=rz, in_=lnz, scale=-1.0,
                                         func=mybir.ActivationFunctionType.Exp)
                    rzb = zpool.tile([64, QW], F32, tag="rzb", name="rzb")
                    nc.gpsimd.partition_broadcast(rzb, rz)
                    dst = yT_sb[jt][po:po + 64, col0:col0 + QW]
                    nc.vector.tensor_mul(dst, yq[q][0:D, :], rzb)
                    nc.vector.tensor_scalar_add(
                        dst, dst, bv_sb[po:po + 64, jt:jt + 1])
        while c_state["m"] < n_tt:
            emit_c_tile()
        bctx.close()

    nc.compile()
    return nc


_CACHED = {}


def _get_program():
    if "nc" not in _CACHED:
        _CACHED["nc"] = build_program()
    return _CACHED["nc"]


def _bv_cols(bv_local):
    """[FL] head-major bias -> [128, HL//2] per-partition columns matching
    the yT layout (head h -> column h//2, rows 64*(h%2)..+64)."""
    arr = np.zeros((128, HL // 2), dtype=np.float32)
    for h in range(HL):
        arr[64 * (h % 2):64 * (h % 2) + 64, h // 2] = bv_local[h * D:(h + 1) * D]
    return arr


def _pack_w128(w):
    """[C, 128] weight slice -> [128, KT*128] with w[k*128+p, f] at
    [p, k*128+f] (lhsT tiles contiguous per k)."""
    return np.ascontiguousarray(
        w.reshape(KT, 128, -1).transpose(1, 0, 2).reshape(128, -1))


def prepare_in_maps(x, w_qkv, b_qkv, w_out):
    in_maps = []
    for core in range(N_CORES):
        b = core // 2
        g = core % 2
        qs, ks, vs = g * FL, C + g * FL, 2 * C + g * FL
        wq = w_qkv[:, qs:qs + FL]
        wk = w_qkv[:, ks:ks + FL]
        wv_ = w_qkv[:, vs:vs + FL]
        wqk8 = np.stack(
            [_pack_w128(wq[:, m * 128:(m + 1) * 128]) for m in range(4)]
            + [_pack_w128(wk[:, m * 128:(m + 1) * 128]) for m in range(4)],
            axis=1)
        bqk8 = np.stack(
            [b_qkv[qs + m * 128:qs + (m + 1) * 128] for m in range(4)]
            + [b_qkv[ks + m * 128:ks + (m + 1) * 128] for m in range(4)],
            axis=1)
        in_maps.append({
            "xT": np.ascontiguousarray(x[b].T).astype(BF),
            "wqk": wqk8.astype(BF),
            "wv": _pack_w128(wv_).astype(BF),
            "wout": np.ascontiguousarray(
                w_out[g * FL:(g + 1) * FL, :]).astype(BF),
            "bqk": np.ascontiguousarray(bqk8, dtype=np.float32),
            "bv": _bv_cols(b_qkv[vs:vs + FL]),
            "tri": np.triu(np.ones((128, 128), dtype=np.float32)).astype(BF),
            "trineg": (np.tril(np.ones((128, 128), dtype=np.float32), -1)
                       * np.float32(-1e9)),
        })
    return in_maps


def gather(results, b_out):
    out = np.empty((B, T, C), dtype=np.float32)
    for b in range(B):
        out[b] = (results[2 * b]["out"].astype(np.float32)
                  + results[2 * b + 1]["out"].astype(np.float32) + b_out)
    return out


def kernel(x, w_qkv, b_qkv, w_out, b_out):
    x = np.asarray(x, dtype=np.float32)
    w_qkv = np.asarray(w_qkv, dtype=np.float32)
    b_qkv = np.asarray(b_qkv, dtype=np.float32)
    w_out = np.asarray(w_out, dtype=np.float32)
    b_out = np.asarray(b_out, dtype=np.float32)

    nc = _get_program()
    in_maps = prepare_in_maps(x, w_qkv, b_qkv, w_out)
    res = run_bass_kernel_spmd(nc, in_maps, core_ids=list(range(N_CORES)))
    return gather(res.results, b_out)


if __name__ == "__main__":
    rng = np.random.default_rng(0)
    inputs = {
        "x": rng.standard_normal((B, T, C), dtype=np.float32),
        "w_qkv": rng.standard_normal((C, 3 * C), dtype=np.float32) * 0.02,
        "b_qkv": np.zeros((3 * C,), dtype=np.float32),
        "w_out": rng.standard_normal((C, C), dtype=np.float32) * 0.02,
        "b_out": np.zeros((C,), dtype=np.float32),
    }
    y = kernel(**inputs)
    print("ok", y.shape, y.dtype)



# revision 8
# speedup vs baseline: 1.2128x; 1.2128x over previous
"""Causal self-attention Trainium2 kernel (v3, bf16, HAM-dense).

Full-model shapes: x [4, 2048, 1024], w_qkv [1024, 3072], b_qkv [3072],
w_out [1024, 1024], b_out [1024].  H=16 heads, D=64.

Sharding: 8 cores = 4 batches x 2 head-groups (tensor parallel).  Each core
computes qkv projection for its 8 heads of its batch, causal attention, and
the partial out-projection (512 of 1024 contraction rows).  The two partials
per batch are summed on the host (the "all-reduce" after out_proj), plus
b_out.

v3 design (vs v2 at ~642us measured):
  - v2's trace showed 64 ACT_TABLE_LOADs (82us of ScalarE) from walrus
    alternating exp_and_others / natural_log sets between the score exp and
    the 1/Z = exp(-ln Z) chain, plus ~60 PE gaps of 4-8us at (head, window)
    boundaries that kept PE_HAM throttled to 1.2 GHz for 413us of the run
    (matmuls avg 436ns vs ~216ns warm).  v3 attacks exactly that:
  - The activation-table map handed to Bacc.insert_act_table_loads is
    filtered to natural_log_exp_and_others (has BOTH exp and ln) so the
    whole kernel uses ONE table load.
  - Emission order is restructured for PE density: qkv projection for
    t<1024 runs first, then the t-half-0 attention windows of all 8 heads
    with the t>=1024 qkv m-tiles interleaved between s-blocks as PE filler,
    then the half-1 windows with out-projection m-tiles as filler.  The PE
    queue (strict FIFO) then always has dependency-free matmuls between
    windows' exp-dependent PVs, so HAM stays at K=8/8.
  - The softmax 1/Z chain is decoupled from PSUM: each yq quarter is
    DVE-copied to SBUF bf16 right after its last PV (freeing the PSUM bank
    after one op), and Ln/Exp/broadcast/mul run lazily off the copy (bf16,
    2x DVE mode).

Layout per core:
  - xT [C, T] bf16 host-transposed; streamed as [128, KT, chunk] tiles.
  - qT [feat, T] (feature-on-partition, 2 heads per 128-tile), kz per-head
    K-padded [128, T] (64 rows k_h + 64 zero rows -> S matmuls contract over
    full 128 partitions).
  - v natural [T, feat] with a ones column per head so PV yields y_un and
    the softmax denominator Z in one PSUM accumulation.
  - scores S^T [s, t] per (s-block, t-half window); diagonal blocks masked
    alternately pre-exp additive (DVE) / post-exp multiplicative (GpSimd).
"""

import sys
from contextlib import ExitStack

import numpy as np

sys.path.insert(0, "/opt/trn_rl_repo")

import ml_dtypes

import concourse.bacc as bacc
import concourse.bass as bass
import concourse.tile as tile
from concourse import mybir
from concourse.bass_utils import run_bass_kernel_spmd

F32 = mybir.dt.float32
F32R = mybir.dt.float32r
BF16 = mybir.dt.bfloat16
BF = ml_dtypes.bfloat16

B, T, C, H = 4, 2048, 1024, 16
D = C // H  # 64
N_CORES = 8
HL = H // 2      # heads per core = 8
FL = HL * D      # local features = 512
KT = C // 128    # 8 contraction tiles


# Pin every activation to the one table set that holds both exp and ln so
# the table-load fixpoint pass emits a single ACT_TABLE_LOAD.  v2 measured
# 64 loads (82us of ScalarE blockage + the PE stalls they cascade into).
_PIN_SET = "natural_log_exp_and_others"
_real_get_tables = bacc.get_activation_tables


def _pinned_get_tables(arch):
    real = _real_get_tables(arch)
    return {name: (fns if name == _PIN_SET else set())
            for name, fns in real.items()}


bacc.get_activation_tables = _pinned_get_tables


# debug bisect flags
EXP_W = 1024          # exp window width (1024 = cross-bank ACT reads)
DIAG_ALT = True       # alternate DVE-additive / GpSimd-mult diagonal masks
LAG = 6               # deferred PV chunks: hides exp latency from the PE


def _chunks(t0, tend, grid=512):
    """Aligned chunks [c0, c0+w) covering [t0, tend), clipped to the global
    `grid` so no chunk crosses a grid (= PSUM bank) boundary."""
    out = []
    while t0 < tend:
        w = min(grid - (t0 % grid), tend - t0)
        out.append((t0, w))
        t0 += w
    return out


def build_program(t_len=T):
    nc = bacc.Bacc(None, target_bir_lowering=False, debug=False)
    TT = t_len
    n_tt = TT // 128

    xT = nc.declare_dram_parameter("xT", [C, TT], BF16, isOutput=False)
    # host-packed: wqk[p, m, k*128+f] = w_m[k*128+p, f]; m 0-3 q, 4-7 k tiles
    wqk = nc.declare_dram_parameter("wqk", [128, 8, KT * 128], BF16,
                                    isOutput=False)
    wv = nc.declare_dram_parameter("wv", [128, KT * FL], BF16, isOutput=False)
    wout = nc.declare_dram_parameter("wout", [FL, C], BF16, isOutput=False)
    bqk = nc.declare_dram_parameter("bqk", [128, 8], F32, isOutput=False)
    bv = nc.declare_dram_parameter("bv", [128, HL // 2], F32, isOutput=False)
    tri = nc.declare_dram_parameter("tri", [128, 128], BF16, isOutput=False)
    trineg = nc.declare_dram_parameter("trineg", [128, 128], F32,
                                       isOutput=False)
    out = nc.declare_dram_parameter("out", [TT, C], F32, isOutput=True)

    HW = min(1024, TT)   # t-half width for exp windows / S psum tiles
    QW = min(512, TT)    # y accumulation quarter width
    n_half = TT // HW

    with tile.TileContext(nc) as tc, ExitStack() as top:
        persist = top.enter_context(tc.tile_pool(name="persist", bufs=1))
        stream = top.enter_context(tc.tile_pool(name="stream", bufs=2))
        upool = top.enter_context(tc.tile_pool(name="u", bufs=LAG + 2))
        zpool = top.enter_context(tc.tile_pool(name="z", bufs=2))
        ycpool = top.enter_context(tc.tile_pool(name="yc", bufs=4))
        obpool = top.enter_context(tc.tile_pool(name="ob", bufs=3))

        wqk_sb = [persist.tile([128, KT * 128], BF16, tag=f"wqk{m}",
                               name=f"wqk{m}") for m in range(8)]
        wv_sb = persist.tile([128, KT * FL], BF16, tag="wv", name="wv_sb")
        qT_sb = [persist.tile([128, TT], BF16, tag=f"qT{j}", name=f"qT{j}")
                 for j in range(4)]
        kz_sb = [persist.tile([128, TT], BF16, tag=f"kz{h}", name=f"kz{h}")
                 for h in range(HL)]
        v_sb = persist.tile([128, n_tt, HL, D + 1], BF16, tag="v", name="v_sb")
        yT_sb = [persist.tile([128, TT], BF16, tag=f"yT{j}", name=f"yT{j}")
                 for j in range(4)]
        wout_sb = [persist.tile([128, C], BF16, tag=f"wo{j}", name=f"wo{j}")
                   for j in range(4)]
        bqk_sb = persist.tile([128, 8], F32, tag="bqk", name="bqk_sb")
        bv_sb = persist.tile([128, HL // 2], F32, tag="bv", name="bv_sb")
        tri_sb = persist.tile([128, 128], BF16, tag="tri", name="tri_sb")
        trineg_sb = persist.tile([128, 128], F32, tag="trineg",
                                 name="trineg_sb")

        # -------- qkv projection over t-chunks; x streamed once -----------
        if TT >= 1024:
            achunks = [(0, 256), (256, 256)] + [
                (c, 512) for c in range(512, TT, 512)]
        else:
            achunks = [(c, 256) for c in range(0, TT, 256)]
        n_pre = len([c for c, _ in achunks if c < HW])  # chunks with t < HW
        xtiles = {}

        def load_chunk(ci):
            c0, ach = achunks[ci]
            xt = stream.tile([128, KT, 512], BF16, tag="x", name=f"x{ci}")
            for k in range(KT):
                nc.sync.dma_start(
                    out=xt[:, k, :ach],
                    in_=xT.rearrange("(k p) t -> p k t", p=128)[:, k,
                                                               c0:c0 + ach])
            xtiles[ci] = xt

        # first matmul needs wqk tile 0 + chunk 0: emit those DMAs first
        nc.sync.dma_start(out=wqk_sb[0], in_=wqk[:, 0, :])
        load_chunk(0)
        for m in range(1, 8):
            nc.sync.dma_start(out=wqk_sb[m], in_=wqk[:, m, :])
        nc.sync.dma_start(out=wv_sb, in_=wv[:])
        nc.sync.dma_start(out=bqk_sb, in_=bqk[:])
        nc.sync.dma_start(out=bv_sb, in_=bv[:])
        nc.sync.dma_start(out=tri_sb, in_=tri[:])
        nc.sync.dma_start(out=trineg_sb, in_=trineg[:])
        for h in range(HL):  # kz zero halves (the K-padding)
            zo = 64 * ((h + 1) % 2)
            nc.vector.memset(kz_sb[h][zo:zo + 64, :], 0.0)
        nc.vector.memset(v_sb[:, :, :, D], 1.0)  # PV ones column -> Z

        def emit_qk_tile(pool, ci, m):
            c0, ach = achunks[ci]
            xt = xtiles[ci]
            ps = pool.tile([128, 512], F32, tag="mm", name="aps")
            for k in range(KT):
                nc.tensor.matmul(
                    ps[:, :ach],
                    wqk_sb[m][:, k * 128:(k + 1) * 128],
                    xt[:, k, :ach],
                    start=(k == 0), stop=(k == KT - 1))
            bias = bqk_sb[:, m:m + 1]
            if m < 4:
                nc.vector.tensor_scalar_add(
                    qT_sb[m][:, c0:c0 + ach], ps[:, :ach], bias)
            else:
                # split k across the two per-head K-padded tiles
                for par in range(2):
                    nc.vector.tensor_scalar_add(
                        kz_sb[(m - 4) * 2 + par][64 * par:64 * par + 64,
                                                 c0:c0 + ach],
                        ps[64 * par:64 * par + 64, :ach],
                        bias[64 * par:64 * par + 64, :])

        def emit_v_tile(pool, ci, sub):
            c0, ach = achunks[ci]
            xt = xtiles[ci]
            ps = pool.tile([128, 512], F32, tag="mm", name="aps")
            for k in range(KT):
                nc.tensor.matmul(
                    ps[:, :FL],
                    xt[:, k, sub * 128:(sub + 1) * 128],
                    wv_sb[:, k * FL:(k + 1) * FL],
                    start=(k == 0), stop=(k == KT - 1))
            it = c0 // 128 + sub
            nc.vector.tensor_copy(
                out=v_sb[:, it, :, 0:D],
                in_=ps[:, :FL].rearrange("p (h d) -> p h d", h=HL))

        # prefix: chunks covering t < HW, pipelined through a 4-bank pool
        actx = ExitStack()
        a_pool = actx.enter_context(
            tc.tile_pool(name="a_psum", bufs=4, space="PSUM"))
        for ci in range(n_pre):
            if ci + 1 < len(achunks):
                load_chunk(ci + 1)
            c0, ach = achunks[ci]
            for m in range(8):  # 4 q tiles then 4 k tiles
                emit_qk_tile(a_pool, ci, m)
            for sub in range(ach // 128):
                emit_v_tile(a_pool, ci, sub)
            xtiles.pop(ci)
        actx.close()
        for j in range(4):  # w_out arrives during early attention
            nc.sync.dma_start(out=wout_sb[j], in_=wout[j * 128:(j + 1) * 128, :])

        # -------- attention; remaining qkv + out-proj interleaved ---------
        bctx = ExitStack()
        s_pool = bctx.enter_context(
            tc.tile_pool(name="s_psum", bufs=2, space="PSUM"))
        y_pool = bctx.enter_context(
            tc.tile_pool(name="y_psum", bufs=2, space="PSUM"))
        aux_pool = bctx.enter_context(
            tc.tile_pool(name="aux_psum", bufs=2, space="PSUM"))

        # filler units: dependency-free PE work threaded between the
        # exp-dependent PV matmuls to keep the PE queue dense (HAM warm).
        fillers = []
        for ci in range(n_pre, len(achunks)):
            load_chunk(ci)
            for m in range(8):
                fillers.append(lambda ci=ci, m=m: emit_qk_tile(aux_pool, ci, m))
            for sub in range(achunks[ci][1] // 128):
                fillers.append(
                    lambda ci=ci, sub=sub: emit_v_tile(aux_pool, ci, sub))

        c_state = {"m": 0}

        def emit_c_tile():
            m = c_state["m"]
            c_state["m"] += 1
            ob = obpool.tile([128, C], F32, tag="ob", name="ob")
            for n in range(C // 512):
                ps = aux_pool.tile([128, 512], F32, tag="mm", name="cps")
                for j in range(4):
                    nc.tensor.matmul(
                        ps,
                        yT_sb[j][:, m * 128:(m + 1) * 128],
                        wout_sb[j][:, n * 512:(n + 1) * 512],
                        start=(j == 0), stop=(j == 3))
                nc.vector.tensor_copy(out=ob[:, n * 512:(n + 1) * 512], in_=ps)
            nc.sync.dma_start(out=out[m * 128:(m + 1) * 128, :], in_=ob)

        # deferred DVE normalize ops (window w's y*(1/Z)+b), drained at the
        # top of window w+1 so a mul waiting on the GpSimd broadcast never
        # blocks the strict-FIFO DVE queue in front of w+1's mask adds.
        norm_pending = []

        def window(h, half):
            jt = h // 2           # q tile / yT tile index
            po = 64 * (h % 2)     # partition offset within the tile pair
            h0 = half * HW
            while norm_pending:
                norm_pending.pop(0)()
            yq = [y_pool.tile([D + 1, QW], F32, tag="y", name=f"yq{q}")
                  for q in range(HW // QW)]
            pend = []

            def emit_pv(s, c0, w, u):
                q = (c0 - h0) // QW
                nc.tensor.matmul(
                    yq[q][:, (c0 - h0) % QW:(c0 - h0) % QW + w],
                    v_sb[:, s, h, :],
                    u[:, c0 - h0:c0 - h0 + w],
                    start=(s == 0), stop=False,
                    skip_group_check=True)

            for s in range(n_tt):
                if s * 128 >= h0 + HW:
                    break
                t0 = max(s * 128, h0)
                loc0 = t0 - h0
                sps = s_pool.tile([128, HW], F32, tag="s", name="sps")
                chunk_list = _chunks(t0, h0 + HW)
                for (c0, w) in chunk_list:
                    nc.tensor.matmul(
                        sps[:, c0 - h0:c0 - h0 + w],
                        kz_sb[h][:, s * 128:(s + 1) * 128],
                        qT_sb[jt][:, c0:c0 + w],
                        start=True, stop=True,
                        skip_group_check=True)
                diag = (s * 128 >= h0)
                use_add = diag and (not DIAG_ALT or s % 2 == 0)
                if use_add:  # pre-exp additive mask (DVE)
                    nc.vector.tensor_add(
                        sps[:, loc0:loc0 + 128], sps[:, loc0:loc0 + 128],
                        trineg_sb)
                u = upool.tile([128, HW], BF16, tag="u", name="u")
                for w0 in range(loc0 // EXP_W * EXP_W, HW, EXP_W):
                    a0, a1 = max(loc0, w0), min(w0 + EXP_W, HW)
                    nc.scalar.activation(
                        out=u[:, a0:a1], in_=sps[:, a0:a1],
                        func=mybir.ActivationFunctionType.Exp,
                        scale=1.0 / np.sqrt(D))
                if diag and not use_add:  # post-exp mul mask (GpSimd)
                    nc.gpsimd.tensor_mul(
                        u[:, loc0:loc0 + 128], u[:, loc0:loc0 + 128],
                        tri_sb)
                for (c0, w) in chunk_list:
                    pend.append((s, c0, w, u))
                while len(pend) > LAG:
                    emit_pv(*pend.pop(0))
                if s % 2 == 1 and fillers:
                    fillers.pop(0)()
            for p in pend:
                emit_pv(*p)

            # deferred softmax normalization, decoupled from PSUM: one DVE
            # copy to SBUF bf16 frees the yq bank; 1/Z = exp(-ln Z) on the
            # Scalar engine (single pinned table set), GpSimd broadcast
            # from partition 0, bf16 DVE mul+bias into yT (deferred).
            for q in range(HW // QW):
                col0 = h0 + q * QW
                yc = ycpool.tile([D + 1, QW], BF16, tag="yc", name="yc")
                nc.vector.tensor_copy(out=yc, in_=yq[q])
                lnz = zpool.tile([1, QW], F32, tag="lnz", name="lnz")
                nc.scalar.activation(out=lnz, in_=yc[D:D + 1, :],
                                     func=mybir.ActivationFunctionType.Ln)
                rz = zpool.tile([1, QW], BF16, tag="rz", name="rz")
                nc.scalar.activation(out=rz, in_=lnz, scale=-1.0,
                                     func=mybir.ActivationFunctionType.Exp)
                rzb = zpool.tile([64, QW], BF16, tag="rzb", name="rzb")
                nc.gpsimd.partition_broadcast(rzb, rz)

                def norm(jt=jt, po=po, col0=col0, yc=yc, rzb=rzb):
                    dst = yT_sb[jt][po:po + 64, col0:col0 + QW]
                    nc.vector.tensor_mul(dst, yc[0:D, :], rzb)
                    nc.vector.tensor_scalar_add(
                        dst, dst, bv_sb[po:po + 64, jt:jt + 1])
                norm_pending.append(norm)

        for half in range(n_half):
            if half == n_half - 1:
                # out-proj m-tiles of completed halves become the filler
                n_ready = (half * HW) // 128
                fillers.extend(
                    [emit_c_tile] * (n_ready - c_state["m"]))
            for h in range(HL):
                window(h, half)
        while norm_pending:
            norm_pending.pop(0)()
        while fillers:
            fillers.pop(0)()
        while c_state["m"] < n_tt:
            emit_c_tile()
        bctx.close()

    nc.compile()
    return nc


_CACHED = {}


def _get_program():
    if "nc" not in _CACHED:
        _CACHED["nc"] = build_program()
    return _CACHED["nc"]


def _bv_cols(bv_local):
    """[FL] head-major bias -> [128, HL//2] per-partition columns matching
    the yT layout (head h -> column h//2, rows 64*(h%2)..+64)."""
    arr = np.zeros((128, HL // 2), dtype=np.float32)
    for h in range(HL):
        arr[64 * (h % 2):64 * (h % 2) + 64, h // 2] = bv_local[h * D:(h + 1) * D]
    return arr


def _pack_w128(w):
    """[C, 128] weight slice -> [128, KT*128] with w[k*128+p, f] at
    [p, k*128+f] (lhsT tiles contiguous per k)."""
    return np.ascontiguousarray(
        w.reshape(KT, 128, -1).transpose(1, 0, 2).reshape(128, -1))


def prepare_in_maps(x, w_qkv, b_qkv, w_out):
    in_maps = []
    for core in range(N_CORES):
        b = core // 2
        g = core % 2
        qs, ks, vs = g * FL, C + g * FL, 2 * C + g * FL
        wq = w_qkv[:, qs:qs + FL]
        wk = w_qkv[:, ks:ks + FL]
        wv_ = w_qkv[:, vs:vs + FL]
        wqk8 = np.stack(
            [_pack_w128(wq[:, m * 128:(m + 1) * 128]) for m in range(4)]
            + [_pack_w128(wk[:, m * 128:(m + 1) * 128]) for m in range(4)],
            axis=1)
        bqk8 = np.stack(
            [b_qkv[qs + m * 128:qs + (m + 1) * 128] for m in range(4)]
            + [b_qkv[ks + m * 128:ks + (m + 1) * 128] for m in range(4)],
            axis=1)
        in_maps.append({
            "xT": np.ascontiguousarray(x[b].T).astype(BF),
            "wqk": wqk8.astype(BF),
            "wv": _pack_w128(wv_).astype(BF),
            "wout": np.ascontiguousarray(
                w_out[g * FL:(g + 1) * FL, :]).astype(BF),
            "bqk": np.ascontiguousarray(bqk8, dtype=np.float32),
            "bv": _bv_cols(b_qkv[vs:vs + FL]),
            "tri": np.triu(np.ones((128, 128), dtype=np.float32)).astype(BF),
            "trineg": (np.tril(np.ones((128, 128), dtype=np.float32), -1)
                       * np.float32(-1e9)),
        })
    return in_maps


def gather(results, b_out):
    out = np.empty((B, T, C), dtype=np.float32)
    for b in range(B):
        out[b] = (results[2 * b]["out"].astype(np.float32)
                  + results[2 * b + 1]["out"].astype(np.float32) + b_out)
    return out


def kernel(x, w_qkv, b_qkv, w_out, b_out):
    x = np.asarray(x, dtype=np.float32)
    w_qkv = np.asarray(w_qkv, dtype=np.float32)
    b_qkv = np.asarray(b_qkv, dtype=np.float32)
    w_out = np.asarray(w_out, dtype=np.float32)
    b_out = np.asarray(b_out, dtype=np.float32)

    nc = _get_program()
    in_maps = prepare_in_maps(x, w_qkv, b_qkv, w_out)
    res = run_bass_kernel_spmd(nc, in_maps, core_ids=list(range(N_CORES)))
    return gather(res.results, b_out)


if __name__ == "__main__":
    rng = np.random.default_rng(0)
    inputs = {
        "x": rng.standard_normal((B, T, C), dtype=np.float32),
        "w_qkv": rng.standard_normal((C, 3 * C), dtype=np.float32) * 0.02,
        "b_qkv": np.zeros((3 * C,), dtype=np.float32),
        "w_out": rng.standard_normal((C, C), dtype=np.float32) * 0.02,
        "b_out": np.zeros((C,), dtype=np.float32),
    }
    y = kernel(**inputs)
    print("ok", y.shape, y.dtype)


# revision 9
# speedup vs baseline: 1.9115x; 1.5761x over previous
"""Causal self-attention Trainium2 kernel (v3, bf16, HAM-dense).

Full-model shapes: x [4, 2048, 1024], w_qkv [1024, 3072], b_qkv [3072],
w_out [1024, 1024], b_out [1024].  H=16 heads, D=64.

Sharding: 8 cores = 4 batches x 2 head-groups (tensor parallel).  Each core
computes qkv projection for its 8 heads of its batch, causal attention, and
the partial out-projection (512 of 1024 contraction rows).  The two partials
per batch are summed on the host (the "all-reduce" after out_proj), plus
b_out.

v3 design (vs v2 at ~642us measured):
  - v2's trace showed 64 ACT_TABLE_LOADs (82us of ScalarE) from walrus
    alternating exp_and_others / natural_log sets between the score exp and
    the 1/Z = exp(-ln Z) chain, plus ~60 PE gaps of 4-8us at (head, window)
    boundaries that kept PE_HAM throttled to 1.2 GHz for 413us of the run
    (matmuls avg 436ns vs ~216ns warm).  v3 attacks exactly that:
  - The activation-table map handed to Bacc.insert_act_table_loads is
    filtered to natural_log_exp_and_others (has BOTH exp and ln) so the
    whole kernel uses ONE table load.
  - Emission order is restructured for PE density: qkv projection for
    t<1024 runs first, then the t-half-0 attention windows of all 8 heads
    with the t>=1024 qkv m-tiles interleaved between s-blocks as PE filler,
    then the half-1 windows with out-projection m-tiles as filler.  The PE
    queue (strict FIFO) then always has dependency-free matmuls between
    windows' exp-dependent PVs, so HAM stays at K=8/8.
  - The softmax 1/Z chain is decoupled from PSUM: each yq quarter is
    DVE-copied to SBUF bf16 right after its last PV (freeing the PSUM bank
    after one op), and Ln/Exp/broadcast/mul run lazily off the copy (bf16,
    2x DVE mode).

Layout per core:
  - xT [C, T] bf16 host-transposed; streamed as [128, KT, chunk] tiles.
  - qT [feat, T] (feature-on-partition, 2 heads per 128-tile), kz per-head
    K-padded [128, T] (64 rows k_h + 64 zero rows -> S matmuls contract over
    full 128 partitions).
  - v natural [T, feat] with a ones column per head so PV yields y_un and
    the softmax denominator Z in one PSUM accumulation.
  - scores S^T [s, t] per (s-block, t-half window); diagonal blocks masked
    alternately pre-exp additive (DVE) / post-exp multiplicative (GpSimd).
"""

import sys
from contextlib import ExitStack

import numpy as np

sys.path.insert(0, "/opt/trn_rl_repo")

import ml_dtypes

import concourse.bacc as bacc
import concourse.bass as bass
import concourse.tile as tile
from concourse import mybir
from concourse.bass_utils import run_bass_kernel_spmd

F32 = mybir.dt.float32
F32R = mybir.dt.float32r
BF16 = mybir.dt.bfloat16
BF = ml_dtypes.bfloat16

B, T, C, H = 4, 2048, 1024, 16
D = C // H  # 64
N_CORES = 8
HL = H // 2      # heads per core = 8
FL = HL * D      # local features = 512
KT = C // 128    # 8 contraction tiles


# Pin every activation to the one table set that holds both exp and ln so
# the table-load fixpoint pass emits a single ACT_TABLE_LOAD.  v2 measured
# 64 loads (82us of ScalarE blockage + the PE stalls they cascade into).
_PIN_SET = "natural_log_exp_and_others"
_real_get_tables = bacc.get_activation_tables


def _pinned_get_tables(arch):
    real = _real_get_tables(arch)
    return {name: (fns if name == _PIN_SET else set())
            for name, fns in real.items()}


bacc.get_activation_tables = _pinned_get_tables


# debug bisect flags
EXP_W = 1024          # exp window width (1024 = cross-bank ACT reads)
# All diagonal masks run pre-exp additive on the DVE: GpSimd then executes
# ONLY partition_broadcast, so it never swaps its custom-op library (v3
# traced 2 UNLOAD_LIB/LOAD_LIB pairs per window at ~6us each between
# broadcast and tensor_mul, stalling the diag PV 12us every window).
DIAG_ALT = False
LAG = 6               # deferred PV chunks: hides exp latency from the PE


def _chunks(t0, tend, grid=512):
    """Aligned chunks [c0, c0+w) covering [t0, tend), clipped to the global
    `grid` so no chunk crosses a grid (= PSUM bank) boundary."""
    out = []
    while t0 < tend:
        w = min(grid - (t0 % grid), tend - t0)
        out.append((t0, w))
        t0 += w
    return out


def build_program(t_len=T):
    nc = bacc.Bacc(None, target_bir_lowering=False, debug=False)
    TT = t_len
    n_tt = TT // 128

    xT = nc.declare_dram_parameter("xT", [C, TT], BF16, isOutput=False)
    # host-packed: wqk[p, m, k*128+f] = w_m[k*128+p, f]; m 0-3 q, 4-7 k tiles
    wqk = nc.declare_dram_parameter("wqk", [128, 8, KT * 128], BF16,
                                    isOutput=False)
    wv = nc.declare_dram_parameter("wv", [128, KT * FL], BF16, isOutput=False)
    wout = nc.declare_dram_parameter("wout", [FL, C], BF16, isOutput=False)
    bqk = nc.declare_dram_parameter("bqk", [128, 8], F32, isOutput=False)
    bv = nc.declare_dram_parameter("bv", [128, HL // 2], F32, isOutput=False)
    tri = nc.declare_dram_parameter("tri", [128, 128], BF16, isOutput=False)
    trineg = nc.declare_dram_parameter("trineg", [128, 128], F32,
                                       isOutput=False)
    out = nc.declare_dram_parameter("out", [TT, C], F32, isOutput=True)

    HW = min(1024, TT)   # t-half width for exp windows / S psum tiles
    QW = min(512, TT)    # y accumulation quarter width
    n_half = TT // HW

    with tile.TileContext(nc) as tc, ExitStack() as top:
        persist = top.enter_context(tc.tile_pool(name="persist", bufs=1))
        stream = top.enter_context(tc.tile_pool(name="stream", bufs=2))
        upool = top.enter_context(tc.tile_pool(name="u", bufs=LAG + 2))
        zpool = top.enter_context(tc.tile_pool(name="z", bufs=2))
        ycpool = top.enter_context(tc.tile_pool(name="yc", bufs=4))
        obpool = top.enter_context(tc.tile_pool(name="ob", bufs=3))

        wqk_sb = [persist.tile([128, KT * 128], BF16, tag=f"wqk{m}",
                               name=f"wqk{m}") for m in range(8)]
        wv_sb = persist.tile([128, KT * FL], BF16, tag="wv", name="wv_sb")
        qT_sb = [persist.tile([128, TT], BF16, tag=f"qT{j}", name=f"qT{j}")
                 for j in range(4)]
        kz_sb = [persist.tile([128, TT], BF16, tag=f"kz{h}", name=f"kz{h}")
                 for h in range(HL)]
        v_sb = persist.tile([128, n_tt, HL, D + 1], BF16, tag="v", name="v_sb")
        yT_sb = [persist.tile([128, TT], BF16, tag=f"yT{j}", name=f"yT{j}")
                 for j in range(4)]
        wout_sb = [persist.tile([128, C], BF16, tag=f"wo{j}", name=f"wo{j}")
                   for j in range(4)]
        bqk_sb = persist.tile([128, 8], F32, tag="bqk", name="bqk_sb")
        bv_sb = persist.tile([128, HL // 2], F32, tag="bv", name="bv_sb")
        tri_sb = persist.tile([128, 128], BF16, tag="tri", name="tri_sb")
        trineg_sb = persist.tile([128, 128], F32, tag="trineg",
                                 name="trineg_sb")

        # -------- qkv projection over t-chunks; x streamed once -----------
        if TT >= 1024:
            achunks = [(0, 256), (256, 256)] + [
                (c, 512) for c in range(512, TT, 512)]
        else:
            achunks = [(c, 256) for c in range(0, TT, 256)]
        n_pre = len([c for c, _ in achunks if c < HW])  # chunks with t < HW
        xtiles = {}

        def load_chunk(ci):
            c0, ach = achunks[ci]
            xt = stream.tile([128, KT, 512], BF16, tag="x", name=f"x{ci}")
            for k in range(KT):
                nc.sync.dma_start(
                    out=xt[:, k, :ach],
                    in_=xT.rearrange("(k p) t -> p k t", p=128)[:, k,
                                                               c0:c0 + ach])
            xtiles[ci] = xt

        # first matmul needs wqk tile 0 + chunk 0: emit those DMAs first
        nc.sync.dma_start(out=wqk_sb[0], in_=wqk[:, 0, :])
        load_chunk(0)
        for m in range(1, 8):
            nc.sync.dma_start(out=wqk_sb[m], in_=wqk[:, m, :])
        nc.sync.dma_start(out=wv_sb, in_=wv[:])
        nc.sync.dma_start(out=bqk_sb, in_=bqk[:])
        nc.sync.dma_start(out=bv_sb, in_=bv[:])
        nc.sync.dma_start(out=tri_sb, in_=tri[:])
        nc.sync.dma_start(out=trineg_sb, in_=trineg[:])
        for h in range(HL):  # kz zero halves (the K-padding)
            zo = 64 * ((h + 1) % 2)
            nc.vector.memset(kz_sb[h][zo:zo + 64, :], 0.0)
        nc.vector.memset(v_sb[:, :, :, D], 1.0)  # PV ones column -> Z

        def emit_qk_tile(pool, ci, m):
            c0, ach = achunks[ci]
            xt = xtiles[ci]
            ps = pool.tile([128, 512], F32, tag="mm", name="aps")
            for k in range(KT):
                nc.tensor.matmul(
                    ps[:, :ach],
                    wqk_sb[m][:, k * 128:(k + 1) * 128],
                    xt[:, k, :ach],
                    start=(k == 0), stop=(k == KT - 1))
            bias = bqk_sb[:, m:m + 1]
            if m < 4:
                nc.vector.tensor_scalar_add(
                    qT_sb[m][:, c0:c0 + ach], ps[:, :ach], bias)
            else:
                # split k across the two per-head K-padded tiles
                for par in range(2):
                    nc.vector.tensor_scalar_add(
                        kz_sb[(m - 4) * 2 + par][64 * par:64 * par + 64,
                                                 c0:c0 + ach],
                        ps[64 * par:64 * par + 64, :ach],
                        bias[64 * par:64 * par + 64, :])

        def emit_v_tile(pool, ci, sub):
            c0, ach = achunks[ci]
            xt = xtiles[ci]
            ps = pool.tile([128, 512], F32, tag="mm", name="aps")
            for k in range(KT):
                nc.tensor.matmul(
                    ps[:, :FL],
                    xt[:, k, sub * 128:(sub + 1) * 128],
                    wv_sb[:, k * FL:(k + 1) * FL],
                    start=(k == 0), stop=(k == KT - 1))
            it = c0 // 128 + sub
            nc.vector.tensor_copy(
                out=v_sb[:, it, :, 0:D],
                in_=ps[:, :FL].rearrange("p (h d) -> p h d", h=HL))

        # prefix: chunks covering t < HW, pipelined through a 4-bank pool
        actx = ExitStack()
        a_pool = actx.enter_context(
            tc.tile_pool(name="a_psum", bufs=4, space="PSUM"))
        for ci in range(n_pre):
            if ci + 1 < len(achunks):
                load_chunk(ci + 1)
            c0, ach = achunks[ci]
            for m in range(8):  # 4 q tiles then 4 k tiles
                emit_qk_tile(a_pool, ci, m)
            for sub in range(ach // 128):
                emit_v_tile(a_pool, ci, sub)
            xtiles.pop(ci)
        actx.close()
        for j in range(4):  # w_out arrives during early attention
            nc.sync.dma_start(out=wout_sb[j], in_=wout[j * 128:(j + 1) * 128, :])

        # -------- attention; remaining qkv + out-proj interleaved ---------
        bctx = ExitStack()
        s_pool = bctx.enter_context(
            tc.tile_pool(name="s_psum", bufs=2, space="PSUM"))
        y_pool = bctx.enter_context(
            tc.tile_pool(name="y_psum", bufs=2, space="PSUM"))
        aux_pool = bctx.enter_context(
            tc.tile_pool(name="aux_psum", bufs=2, space="PSUM"))

        # filler units: dependency-free PE work threaded between the
        # exp-dependent PV matmuls to keep the PE queue dense (HAM warm).
        fillers = []
        for ci in range(n_pre, len(achunks)):
            load_chunk(ci)
            for m in range(8):
                fillers.append(lambda ci=ci, m=m: emit_qk_tile(aux_pool, ci, m))
            for sub in range(achunks[ci][1] // 128):
                fillers.append(
                    lambda ci=ci, sub=sub: emit_v_tile(aux_pool, ci, sub))

        c_state = {"m": 0}

        def emit_c_tile():
            m = c_state["m"]
            c_state["m"] += 1
            ob = obpool.tile([128, C], F32, tag="ob", name="ob")
            for n in range(C // 512):
                ps = aux_pool.tile([128, 512], F32, tag="mm", name="cps")
                for j in range(4):
                    nc.tensor.matmul(
                        ps,
                        yT_sb[j][:, m * 128:(m + 1) * 128],
                        wout_sb[j][:, n * 512:(n + 1) * 512],
                        start=(j == 0), stop=(j == 3))
                nc.vector.tensor_copy(out=ob[:, n * 512:(n + 1) * 512], in_=ps)
            nc.sync.dma_start(out=out[m * 128:(m + 1) * 128, :], in_=ob)

        # deferred DVE normalize ops (window w's y*(1/Z)+b), drained at the
        # top of window w+1 so a mul waiting on the GpSimd broadcast never
        # blocks the strict-FIFO DVE queue in front of w+1's mask adds.
        norm_pending = []

        def window(h, half):
            jt = h // 2           # q tile / yT tile index
            po = 64 * (h % 2)     # partition offset within the tile pair
            h0 = half * HW
            while norm_pending:
                norm_pending.pop(0)()
            yq = [y_pool.tile([D + 1, QW], F32, tag="y", name=f"yq{q}")
                  for q in range(HW // QW)]
            pend = []

            def emit_pv(s, c0, w, u):
                q = (c0 - h0) // QW
                nc.tensor.matmul(
                    yq[q][:, (c0 - h0) % QW:(c0 - h0) % QW + w],
                    v_sb[:, s, h, :],
                    u[:, c0 - h0:c0 - h0 + w],
                    start=(s == 0), stop=False,
                    skip_group_check=True)

            for s in range(n_tt):
                if s * 128 >= h0 + HW:
                    break
                t0 = max(s * 128, h0)
                loc0 = t0 - h0
                sps = s_pool.tile([128, HW], F32, tag="s", name="sps")
                chunk_list = _chunks(t0, h0 + HW)
                for (c0, w) in chunk_list:
                    nc.tensor.matmul(
                        sps[:, c0 - h0:c0 - h0 + w],
                        kz_sb[h][:, s * 128:(s + 1) * 128],
                        qT_sb[jt][:, c0:c0 + w],
                        start=True, stop=True,
                        skip_group_check=True)
                diag = (s * 128 >= h0)
                use_add = diag and (not DIAG_ALT or s % 2 == 0)
                if use_add:  # pre-exp additive mask (DVE)
                    nc.vector.tensor_add(
                        sps[:, loc0:loc0 + 128], sps[:, loc0:loc0 + 128],
                        trineg_sb)
                u = upool.tile([128, HW], BF16, tag="u", name="u")
                for w0 in range(loc0 // EXP_W * EXP_W, HW, EXP_W):
                    a0, a1 = max(loc0, w0), min(w0 + EXP_W, HW)
                    nc.scalar.activation(
                        out=u[:, a0:a1], in_=sps[:, a0:a1],
                        func=mybir.ActivationFunctionType.Exp,
                        scale=1.0 / np.sqrt(D))
                if diag and not use_add:  # post-exp mul mask (GpSimd)
                    nc.gpsimd.tensor_mul(
                        u[:, loc0:loc0 + 128], u[:, loc0:loc0 + 128],
                        tri_sb)
                for (c0, w) in chunk_list:
                    pend.append((s, c0, w, u))
                while len(pend) > LAG:
                    emit_pv(*pend.pop(0))
                if s % 2 == 1 and fillers:
                    fillers.pop(0)()
            for p in pend:
                emit_pv(*p)

            # deferred softmax normalization, decoupled from PSUM: one DVE
            # copy to SBUF bf16 frees the yq bank; 1/Z = exp(-ln Z) on the
            # Scalar engine (single pinned table set), GpSimd broadcast
            # from partition 0, bf16 DVE mul+bias into yT (deferred).
            for q in range(HW // QW):
                col0 = h0 + q * QW
                yc = ycpool.tile([D + 1, QW], BF16, tag="yc", name="yc")
                nc.vector.tensor_copy(out=yc, in_=yq[q])
                lnz = zpool.tile([1, QW], F32, tag="lnz", name="lnz")
                nc.scalar.activation(out=lnz, in_=yc[D:D + 1, :],
                                     func=mybir.ActivationFunctionType.Ln)
                rz = zpool.tile([1, QW], BF16, tag="rz", name="rz")
                nc.scalar.activation(out=rz, in_=lnz, scale=-1.0,
                                     func=mybir.ActivationFunctionType.Exp)
                rzb = zpool.tile([64, QW], BF16, tag="rzb", name="rzb")
                nc.gpsimd.partition_broadcast(rzb, rz)

                def norm(jt=jt, po=po, col0=col0, yc=yc, rzb=rzb):
                    dst = yT_sb[jt][po:po + 64, col0:col0 + QW]
                    nc.vector.tensor_mul(dst, yc[0:D, :], rzb)
                    nc.vector.tensor_scalar_add(
                        dst, dst, bv_sb[po:po + 64, jt:jt + 1])
                norm_pending.append(norm)

        for half in range(n_half):
            if half == n_half - 1:
                # out-proj m-tiles of completed halves become the filler
                n_ready = (half * HW) // 128
                fillers.extend(
                    [emit_c_tile] * (n_ready - c_state["m"]))
            for h in range(HL):
                window(h, half)
        while norm_pending:
            norm_pending.pop(0)()
        while fillers:
            fillers.pop(0)()
        while c_state["m"] < n_tt:
            emit_c_tile()
        bctx.close()

    nc.compile()
    return nc


_CACHED = {}


def _get_program():
    if "nc" not in _CACHED:
        _CACHED["nc"] = build_program()
    return _CACHED["nc"]


def _bv_cols(bv_local):
    """[FL] head-major bias -> [128, HL//2] per-partition columns matching
    the yT layout (head h -> column h//2, rows 64*(h%2)..+64)."""
    arr = np.zeros((128, HL // 2), dtype=np.float32)
    for h in range(HL):
        arr[64 * (h % 2):64 * (h % 2) + 64, h // 2] = bv_local[h * D:(h + 1) * D]
    return arr


def _pack_w128(w):
    """[C, 128] weight slice -> [128, KT*128] with w[k*128+p, f] at
    [p, k*128+f] (lhsT tiles contiguous per k)."""
    return np.ascontiguousarray(
        w.reshape(KT, 128, -1).transpose(1, 0, 2).reshape(128, -1))


def prepare_in_maps(x, w_qkv, b_qkv, w_out):
    in_maps = []
    for core in range(N_CORES):
        b = core // 2
        g = core % 2
        qs, ks, vs = g * FL, C + g * FL, 2 * C + g * FL
        wq = w_qkv[:, qs:qs + FL]
        wk = w_qkv[:, ks:ks + FL]
        wv_ = w_qkv[:, vs:vs + FL]
        wqk8 = np.stack(
            [_pack_w128(wq[:, m * 128:(m + 1) * 128]) for m in range(4)]
            + [_pack_w128(wk[:, m * 128:(m + 1) * 128]) for m in range(4)],
            axis=1)
        bqk8 = np.stack(
            [b_qkv[qs + m * 128:qs + (m + 1) * 128] for m in range(4)]
            + [b_qkv[ks + m * 128:ks + (m + 1) * 128] for m in range(4)],
            axis=1)
        in_maps.append({
            "xT": np.ascontiguousarray(x[b].T).astype(BF),
            "wqk": wqk8.astype(BF),
            "wv": _pack_w128(wv_).astype(BF),
            "wout": np.ascontiguousarray(
                w_out[g * FL:(g + 1) * FL, :]).astype(BF),
            "bqk": np.ascontiguousarray(bqk8, dtype=np.float32),
            "bv": _bv_cols(b_qkv[vs:vs + FL]),
            "tri": np.triu(np.ones((128, 128), dtype=np.float32)).astype(BF),
            "trineg": (np.tril(np.ones((128, 128), dtype=np.float32), -1)
                       * np.float32(-1e9)),
        })
    return in_maps


def gather(results, b_out):
    out = np.empty((B, T, C), dtype=np.float32)
    for b in range(B):
        out[b] = (results[2 * b]["out"].astype(np.float32)
                  + results[2 * b + 1]["out"].astype(np.float32) + b_out)
    return out


def kernel(x, w_qkv, b_qkv, w_out, b_out):
    x = np.asarray(x, dtype=np.float32)
    w_qkv = np.asarray(w_qkv, dtype=np.float32)
    b_qkv = np.asarray(b_qkv, dtype=np.float32)
    w_out = np.asarray(w_out, dtype=np.float32)
    b_out = np.asarray(b_out, dtype=np.float32)

    nc = _get_program()
    in_maps = prepare_in_maps(x, w_qkv, b_qkv, w_out)
    res = run_bass_kernel_spmd(nc, in_maps, core_ids=list(range(N_CORES)))
    return gather(res.results, b_out)


if __name__ == "__main__":
    rng = np.random.default_rng(0)
    inputs = {
        "x": rng.standard_normal((B, T, C), dtype=np.float32),
        "w_qkv": rng.standard_normal((C, 3 * C), dtype=np.float32) * 0.02,
        "b_qkv": np.zeros((3 * C,), dtype=np.float32),
        "w_out": rng.standard_normal((C, C), dtype=np.float32) * 0.02,
        "b_out": np.zeros((C,), dtype=np.float32),
    }
    y = kernel(**inputs)
    print("ok", y.shape, y.dtype)
